# revision 1
# baseline (speedup 1.0000x reference)
"""Caser (dense_cnn) Trainium2 kernel: batch-parallel over 8 NeuronCores.

Strategy
--------
Data-parallel over batch (512 rows/core); conv/FC weights folded on host
into two matmuls; item/user/W2 tables replicated per core and gathered
on-device with SWDGE indirect DMA (per-partition offset lists).

Per core:
  stage 1: gather item/user embeddings (one row/partition per SWDGE
           indirect DMA -- the only gather shape this HW path honors),
           PE-transpose, two folded matmuls
           -> xcaug = [relu(fc1) | uemb | 1.0 | 0] f32
  stage 2: gather W2cat rows (w2|b2|0 packed f32) per (b, t); VectorE
           broadcast-mult + tree reduce -> res[b, t]

All-f32 throughout (bit-exact vs the f32 reference up to reduction
order). The cost is dominated by the 424 per-core indirect-gather
instructions (SWDGE instruction overhead), which the DVE mult+reduce
overlaps.
"""

import numpy as np
import ml_dtypes

from concourse import bass, bacc, mybir, tile
from concourse.bass_utils import run_bass_kernel_spmd
from concourse.masks import make_identity

B, L, D, NH, NV, T = 4096, 5, 128, 16, 4, 100
NITEMS = NUSERS = 100000
NCORES = 8
BC = B // NCORES          # 512 batch rows per core
NBLK = BC // 128          # 4 blocks of 128
ROW = 2 * D + 2           # w2cat row: 256 w2 | b2 | 0-pad
TCH = 50                  # t-chunk size for stage 2
NCH = T // TCH
FC1 = 752
HOR = 240                 # total horizontal conv outputs (16 * (5+4+3+2+1))
XW = D + HOR              # 368 columns in the fused stage-1 matmul

F32 = mybir.dt.float32
BF16 = mybir.dt.bfloat16
I32 = mybir.dt.int32


def _build():
    nc = bacc.Bacc(None, target_bir_lowering=False)

    w2cat = nc.declare_dram_parameter("w2cat", [NITEMS, ROW], F32, isOutput=False)
    item_t = nc.declare_dram_parameter("item_table", [NITEMS, D], F32, isOutput=False)
    user_t = nc.declare_dram_parameter("user_table", [NUSERS, D], F32, isOutput=False)
    seq_idx = nc.declare_dram_parameter("seq_idx", [128, NBLK * L], I32, isOutput=False)
    user_idx = nc.declare_dram_parameter("user_idx", [128, NBLK], I32, isOutput=False)
    items_idx = nc.declare_dram_parameter("items_idx", [128, NBLK * T], I32, isOutput=False)
    cc_d = nc.declare_dram_parameter("cc", [128, L, XW], F32, isOutput=False)
    w1b_d = nc.declare_dram_parameter("w1b", [120, 2, D], F32, isOutput=False)
    crow_d = nc.declare_dram_parameter("crow", [1, XW], F32, isOutput=False)
    res_d = nc.declare_dram_parameter("res", [BC, T], F32, isOutput=True)

    with tile.TileContext(nc) as tc:
        with (
            tc.tile_pool(name="const", bufs=1) as cp,
            tc.tile_pool(name="work", bufs=2) as wp,
            tc.tile_pool(name="gather", bufs=3) as gp,
            tc.tile_pool(name="psum", bufs=2, space="PSUM") as pp,
        ):
            ident = cp.tile([128, 128], F32)
            make_identity(nc, ident[:])
            ones = cp.tile([1, 128], F32)
            nc.vector.memset(ones[:], 1.0)

            cc_sb = cp.tile([128, L, XW], F32)
            nc.sync.dma_start(out=cc_sb[:], in_=cc_d[:])
            w1b_sb = cp.tile([120, 2, D], F32)
            nc.sync.dma_start(out=w1b_sb[:], in_=w1b_d[:])
            crow_sb = cp.tile([1, XW], F32)
            nc.sync.dma_start(out=crow_sb[:], in_=crow_d[:])

            seqidx_sb = cp.tile([128, NBLK * L], I32)
            nc.sync.dma_start(out=seqidx_sb[:], in_=seq_idx[:])
            useridx_sb = cp.tile([128, NBLK], I32)
            nc.sync.dma_start(out=useridx_sb[:], in_=user_idx[:])
            itemsidx_sb = cp.tile([128, NBLK * T], I32)
            nc.sync.dma_start(out=itemsidx_sb[:], in_=items_idx[:])

            # --- embedding gathers (per-partition offset lists) ---
            emb_sb = cp.tile([128, NBLK * L, D], F32)
            for j in range(NBLK * L):
                nc.gpsimd.indirect_dma_start(
                    out=emb_sb[:, j, :],
                    out_offset=None,
                    in_=item_t[:],
                    in_offset=bass.IndirectOffsetOnAxis(ap=seqidx_sb[:, j : j + 1], axis=0),
                )
            uemb_sb = cp.tile([128, NBLK, D], F32)
            for j in range(NBLK):
                nc.gpsimd.indirect_dma_start(
                    out=uemb_sb[:, j, :],
                    out_offset=None,
                    in_=user_t[:],
                    in_offset=bass.IndirectOffsetOnAxis(ap=useridx_sb[:, j : j + 1], axis=0),
                )

            xcaug = []
            res_sb = []
            for blk in range(NBLK):
                xcaug.append(
                    cp.tile([128, ROW], F32, tag=f"xcaug{blk}", name=f"xcaug{blk}")
                )
                res_sb.append(
                    cp.tile([128, T], F32, tag=f"res{blk}", name=f"res{blk}")
                )

            # ---------------- stage 1 ----------------
            for blk in range(NBLK):
                embT_ps = pp.tile([128, L * 128], F32, tag="embT")
                for t in range(L):
                    nc.tensor.transpose(
                        out=embT_ps[:, t * 128 : (t + 1) * 128],
                        in_=emb_sb[:, blk * L + t, :],
                        identity=ident[:],
                    )
                embT_sb = wp.tile([128, L * 128], F32, tag="embT_sb")
                nc.scalar.copy(out=embT_sb[:], in_=embT_ps[:])

                x_ps = pp.tile([128, XW], F32, tag="xps")
                for t in range(L):
                    nc.tensor.matmul(
                        out=x_ps[:],
                        lhsT=embT_sb[:, t * 128 : (t + 1) * 128],
                        rhs=cc_sb[:, t, :],
                        start=(t == 0),
                        stop=False,
                        skip_group_check=True,
                    )
                nc.tensor.matmul(
                    out=x_ps[:],
                    lhsT=ones[0:1, :],
                    rhs=crow_sb[0:1, :],
                    start=False,
                    stop=False,
                    skip_group_check=True,
                )
                h_sb = wp.tile([128, HOR], F32, tag="h_sb")
                nc.scalar.activation(
                    out=h_sb[:],
                    in_=x_ps[:, D : D + HOR],
                    func=mybir.ActivationFunctionType.Relu,
                )
                hT_ps = pp.tile([120, 256], F32, tag="hT")
                for c2 in range(2):
                    nc.tensor.transpose(
                        out=hT_ps[:, c2 * 128 : c2 * 128 + 128],
                        in_=h_sb[:, c2 * 120 : (c2 + 1) * 120],
                        identity=ident[:],
                    )
                hT_sb = wp.tile([120, 256], F32, tag="hT_sb")
                nc.scalar.copy(out=hT_sb[:], in_=hT_ps[:])
                for c2 in range(2):
                    nc.tensor.matmul(
                        out=x_ps[:, 0:D],
                        lhsT=hT_sb[:, c2 * 128 : c2 * 128 + 128],
                        rhs=w1b_sb[:, c2, :],
                        start=False,
                        stop=(c2 == 1),
                        skip_group_check=True,
                    )
                # xcaug = [relu(x) | uemb | 1.0 | 0.0]
                nc.scalar.activation(
                    out=xcaug[blk][:, 0:D],
                    in_=x_ps[:, 0:D],
                    func=mybir.ActivationFunctionType.Relu,
                )
                nc.vector.tensor_copy(xcaug[blk][:, D : 2 * D], uemb_sb[:, blk, :])
                nc.vector.memset(xcaug[blk][:, 2 * D : 2 * D + 1], 1.0)
                nc.vector.memset(xcaug[blk][:, 2 * D + 1 : ROW], 0.0)

            # ---------------- stage 2 ----------------
            for blk in range(NBLK):
                for h in range(NCH):
                    c0 = blk * T + h * TCH
                    w2_sb = gp.tile([128, TCH, ROW], F32, tag="w2")
                    for t in range(TCH):
                        nc.gpsimd.indirect_dma_start(
                            out=w2_sb[:, t, :],
                            out_offset=None,
                            in_=w2cat[:],
                            in_offset=bass.IndirectOffsetOnAxis(
                                ap=itemsidx_sb[:, c0 + t : c0 + t + 1], axis=0
                            ),
                        )
                    xa = xcaug[blk][:]
                    xb = bass.AP(xa.tensor, xa.offset, [xa.ap[0], [0, TCH], [1, ROW]])
                    nc.vector.tensor_tensor(
                        out=w2_sb[:], in0=w2_sb[:], in1=xb, op=mybir.AluOpType.mult
                    )
                    w = 128
                    while w >= 1:
                        nc.vector.tensor_tensor(
                            out=w2_sb[:, :, 0:w],
                            in0=w2_sb[:, :, 0:w],
                            in1=w2_sb[:, :, w : 2 * w],
                            op=mybir.AluOpType.add,
                        )
                        w //= 2
                    wa = w2_sb[:]
                    dot = bass.AP(wa.tensor, wa.offset, [wa.ap[0], [ROW, TCH]])
                    b2t = bass.AP(wa.tensor, wa.offset + 2 * D, [wa.ap[0], [ROW, TCH]])
                    nc.vector.tensor_tensor(
                        out=res_sb[blk][:, h * TCH : (h + 1) * TCH],
                        in0=dot,
                        in1=b2t,
                        op=mybir.AluOpType.add,
                    )
                nc.sync.dma_start(
                    out=res_d[blk * 128 : (blk + 1) * 128, :], in_=res_sb[blk][:]
                )

    nc.finalize()
    return nc


def _fold_weights(Wv, bv, Wh, bh, W1, b1):
    Wv = np.asarray(Wv, np.float32)
    bv = np.asarray(bv, np.float32)
    Wh = np.asarray(Wh, np.float32)
    bh = np.asarray(bh, np.float32)
    W1 = np.asarray(W1, np.float32)
    b1 = np.asarray(b1, np.float32)

    W1a = W1[: NV * D].reshape(NV, D, D)          # [v, d, m]
    C = np.einsum("vt,vdm->tdm", Wv, W1a)          # [t, d, m]
    const_bv = np.einsum("v,vdm->m", bv, W1a)      # [m]

    H = np.zeros((L, D, HOR), np.float32)          # [t, d, j]
    bh_rep = np.zeros(HOR, np.float32)
    off = 0
    for l in range(1, L + 1):
        lout = L - l + 1
        for f in range(NH):
            for tau in range(lout):
                j = off + f * lout + tau
                bh_rep[j] = bh[l - 1, f]
                for s in range(l):
                    H[tau + s, :, j] = Wh[l - 1, f, s, :]
        off += NH * lout

    CC = np.concatenate([C.reshape(L * D, D), H.reshape(L * D, HOR)], axis=1)
    cc = CC.reshape(L, 128, XW).transpose(1, 0, 2).copy()      # [128, t, n]

    W1b = W1[NV * D :]                              # [240, 128]
    w1b = W1b.reshape(2, 120, D).transpose(1, 0, 2).copy()     # [120, c2, m]

    crow = np.zeros((1, XW), np.float32)
    crow[0, :D] = b1 + const_bv
    crow[0, D:] = bh_rep
    return cc, w1b, crow


_NC = None


def _prepare(inputs):
    seq = np.asarray(inputs["seq"]).astype(np.int32)
    user = np.asarray(inputs["user"]).astype(np.int32)
    items = np.asarray(inputs["items"]).astype(np.int32)
    item_table = np.asarray(inputs["item_table"], np.float32)
    user_table = np.asarray(inputs["user_table"], np.float32)
    W2_table = np.asarray(inputs["W2_table"], np.float32)
    b2_table = np.asarray(inputs["b2_table"], np.float32)

    cc, w1b, crow = _fold_weights(
        inputs["Wv"], inputs["bv"], inputs["Wh"], inputs["bh"],
        inputs["W1"], inputs["b1"],
    )

    w2cat = np.zeros((NITEMS, ROW), np.float32)
    w2cat[:, : 2 * D] = W2_table
    w2cat[:, 2 * D] = b2_table[:, 0]

    in_maps = []
    for c in range(NCORES):
        s = slice(c * BC, (c + 1) * BC)
        # [p, blk, ...] layouts: batch row = c*BC + blk*128 + p
        sq = seq[s].reshape(NBLK, 128, L).transpose(1, 0, 2).reshape(128, NBLK * L)
        us = user[s].reshape(NBLK, 128).transpose(1, 0).copy()
        it = items[s].reshape(NBLK, 128, T).transpose(1, 0, 2).reshape(128, NBLK * T)
        in_maps.append(
            {
                "w2cat": w2cat,
                "item_table": item_table,
                "user_table": user_table,
                "seq_idx": np.ascontiguousarray(sq),
                "user_idx": np.ascontiguousarray(us),
                "items_idx": np.ascontiguousarray(it),
                "cc": cc,
                "w1b": w1b,
                "crow": crow,
            }
        )
    return in_maps


def kernel(**inputs):
    global _NC
    if _NC is None:
        _NC = _build()
    in_maps = _prepare(inputs)
    r = run_bass_kernel_spmd(_NC, in_maps, list(range(NCORES)))
    out = np.concatenate([r.results[c]["res"] for c in range(NCORES)], axis=0)
    return out.astype(np.float32)


def run_traced(inputs, **kw):
    """Profiled run; returns BassKernelResults with exec_time_ns."""
    global _NC
    if _NC is None:
        _NC = _build()
    in_maps = _prepare(inputs)
    return run_bass_kernel_spmd(_NC, in_maps, list(range(NCORES)), trace=True, **kw)

